# revision 1
# baseline (speedup 1.0000x reference)
"""BitLinear (1.58-bit) kernel for Trainium2, 8-core data-parallel SPMD.

Reference op: out = sign(x) @ ternarize(W).T where
  ternarize(W) = sign(W) * min(round(|W| / gamma), 1), gamma = mean(|W|) + 1e-6.

Strategy (per sharding hint: data-parallel over batch*seq, replicate ternary W):
  - Host: ternarize W once (the "small 2048x2048 ternary weight" of the hint),
    transpose to [in, out] and pack as fp8e4 (values -1/0/+1 are exact in fp8).
    Shard x by rows (batch*seq) across the 8 cores; pre-transpose each shard to
    [in, rows] so the contraction dim lands on SBUF partitions with contiguous
    DMA lines.
  - Device (per core): DMA x^T chunks (f32), compute sign() on the Scalar
    engine straight to fp8, then a dense fp8 DoubleRow matmul (2 MACs/cell/cyc)
    accumulating in PSUM f32.  Products are +-1 and row sums <= 2048 so fp32
    accumulation is exact.
  - Host: concatenate the 8 output shards.

Layout: contraction index i in [0, 2048) is split as i = kc*256 + j*128 + p
(kc = 256-wide chunk, j = DoubleRow pair slot, p = SBUF partition).  Both
operands are stored [128, KC, 2, N] in SBUF and sliced to the 3D
[128 part, 2, N] APs that MatmulPerfMode.DoubleRow requires.
"""

import numpy as np
import ml_dtypes

import concourse.bass as bass
import concourse.bacc as bacc
import concourse.mybir as mybir
from concourse.tile import TileContext
from concourse.bass_utils import run_bass_kernel_spmd

FP8 = ml_dtypes.float8_e4m3  # maps to mybir.dt.float8e4

N_CORES = 8
EPS = 1e-6

# Full-problem shapes (hardcoded per harness contract).
B, S, I_DIM, O_DIM = 4, 4096, 2048, 2048
M_TOT = B * S                 # 16384 rows
M_PER = M_TOT // N_CORES      # 2048 rows per core


def _x_groups(mt: int) -> list:
    """m-block DMA groups, deadline-ordered (first blocks needed first)."""
    return [(0, 2), (2, 8), (8, mt)] if mt >= 16 else [(0, mt)]


def _w_groups(kc: int, o_dim: int) -> list:
    """(k0, k1, o0, o1) DMA groups for the quantized weight."""
    if kc >= 8:
        return [(0, 1, 0, o_dim), (1, 2, 0, o_dim), (2, 4, 0, o_dim),
                (4, 6, 0, o_dim), (6, 8, 0, o_dim)]
    return [(0, kc, 0, o_dim)]


def build_program(m_per: int, k_dim: int, o_dim: int) -> bass.Bass:
    """Per-core SPMD program: out[m, o] = sign(x)[m, :] @ Wq[o, :].T.

    DRAM inputs (flat u8, concatenated per-DMA-group partition-major blocks):
      xt : x^T bf16 high bytes, blocks [(b1-b0), ...] as [128p, b, kc, 2, 128m]
           with i = kc*256 + j*128 + p, m = mb*128 + mi
      wt : ternary Wq^T as fp8e4 bytes, blocks [128p, k, 2, o_dim]
    DRAM output:
      out: [m_per, o_dim] f16 (values are integers <= 2048, exact)
    """
    KC = k_dim // 256          # 256-wide contraction chunks
    MT = m_per // 128          # output row tiles
    OT = o_dim // 512          # output col chunks (one PSUM bank each)
    assert k_dim % 256 == 0 and m_per % 128 == 0 and o_dim % 512 == 0

    # Bacc (not plain Bass): its finalize() runs generate_event_semaphores,
    # which splits multi-waits to the HW limit of 1 wait per instruction.
    nc = bacc.Bacc()
    # x travels as the high byte of its bf16 encoding (sign + 7 exponent
    # bits) — a pure byte-slice of the input that halves x traffic and is
    # exact for the sign() this op needs.  m-block-major layout: the first
    # output tile only needs block 0, so the x deadlines spread out across
    # the whole kernel instead of all landing in the first ~14us.
    xg = _x_groups(MT)
    wg = _w_groups(KC, o_dim)
    x_total = MT * 128 * KC * 2 * 128
    w_total = KC * 128 * 2 * o_dim
    xt = nc.declare_dram_parameter(
        "xt", [x_total], mybir.dt.uint8, isOutput=False)
    wt = nc.declare_dram_parameter(
        "wt", [w_total], mybir.dt.uint8, isOutput=False)
    # f16 output: every value is an integer in [-2048, 2048], exact in f16;
    # the host casts back to f32.  Halves the output DMA traffic.
    out = nc.declare_dram_parameter(
        "out", [m_per, o_dim], mybir.dt.float16, isOutput=True)

    with TileContext(nc) as tc:
        with (
            tc.tile_pool(name="wq", bufs=1) as wq_pool,
            tc.tile_pool(name="xs", bufs=1) as xs_pool,
            tc.tile_pool(name="xraw", bufs=1) as xraw_pool,
            tc.tile_pool(name="psum", bufs=4, space="PSUM") as psum_pool,
            tc.tile_pool(name="osb", bufs=6) as out_pool,
        ):
            # Staging is m-block-major: [128, MT, KC, 2, 128].  Write-once
            # (bufs=1, disjoint slices) keeps every HWDGE DMA at <=1
            # embedded sync wait (walrus limit).
            xr_sb = xraw_pool.tile([128, MT, KC, 2, 128], mybir.dt.uint8)
            xs_sb = xs_pool.tile([128, MT, KC, 2, 128], mybir.dt.uint8)
            wq_sb = wq_pool.tile([128, KC, 2, o_dim], mybir.dt.float8e4)

            # Each HWDGE DMA pays ~0.7us issue/receipt overhead, transfers
            # serialize per queue, and strided DRAM reads run below line
            # rate — so every group is a contiguous flat slice (per-group
            # partition-major host packing), deadline-ordered:
            #   SP queue:  x mb0-1 | wq kc4-5 | wq kc6-7 | x mb2-7 | x mb8-15
            #   ACT queue: wq kc0 | wq kc1 | wq kc2-3
            x_off = {}
            off = 0
            for b0, b1 in xg:
                x_off[(b0, b1)] = off
                off += (b1 - b0) * 128 * KC * 2 * 128
            w_off = {}
            off = 0
            for g in wg:
                w_off[g] = off
                off += (g[1] - g[0]) * 128 * 2 * (g[3] - g[2])

            def dma_x(eng, b0, b1):
                sz = (b1 - b0) * 128 * KC * 2 * 128
                o0 = x_off[(b0, b1)]
                eng.dma_start(
                    out=xr_sb[:, b0:b1],
                    in_=xt[o0:o0 + sz].rearrange("(p r) -> p r", p=128))

            def dma_w(eng, g):
                k0, k1, c0, c1 = g
                sz = (k1 - k0) * 128 * 2 * (c1 - c0)
                o0 = w_off[g]
                eng.dma_start(
                    out=wq_sb[:, k0:k1, :, c0:c1].bitcast(mybir.dt.uint8),
                    in_=wt[o0:o0 + sz].rearrange("(p r) -> p r", p=128))

            if MT >= 16 and len(wg) == 5:
                dma_x(nc.sync, 0, 2)
                dma_w(nc.sync, wg[3])     # kc 4-5
                dma_w(nc.sync, wg[4])     # kc 6-7
                dma_x(nc.sync, 2, 8)
                dma_x(nc.sync, 8, MT)
                dma_w(nc.scalar, wg[0])   # kc 0
                dma_w(nc.scalar, wg[1])   # kc 1
                dma_w(nc.scalar, wg[2])   # kc 2-3
            else:  # small (sim) shapes
                for b0, b1 in xg:
                    dma_x(nc.sync, b0, b1)
                for g in wg:
                    dma_w(nc.scalar, g)

            # One-pass sign to fp8 {+1, -1} on DVE, 4 bytes per lane-cycle:
            # view the hi bytes as u32 and compute (v & 0x80808080) |
            # 0x38383838 — each byte becomes the fp8e4 encoding of sign(x).
            # Per m-block, matching both the DMA granularity and the matmul
            # consumption order (subtile deps).
            for mb in range(MT):
                src = xr_sb[:, mb]
                dst = xs_sb[:, mb]
                nc.vector.tensor_scalar(
                    out=dst.bitcast(mybir.dt.uint32),
                    in0=src.bitcast(mybir.dt.uint32),
                    scalar1=0x80808080, scalar2=0x38383838,
                    op0=mybir.AluOpType.bitwise_and,
                    op1=mybir.AluOpType.bitwise_or)

            # PE warmup: dummy matmuls on memset scratch keep the PE busy
            # through the HAM activity window while the first x chunk lands,
            # so real matmuls start at the 2.4 GHz warm clock.
            wu_a = wq_pool.tile([128, 2, 128], mybir.dt.float8e4)
            wu_b = wq_pool.tile([128, 2, 512], mybir.dt.float8e4)
            nc.gpsimd.memset(wu_a, 0.0)
            nc.gpsimd.memset(wu_b, 0.0)
            wu_ps = psum_pool.tile([128, 512], mybir.dt.float32,
                                   name="wu_ps", tag="ps")
            for _ in range(16):
                nc.tensor.matmul(wu_ps, wu_a, wu_b, start=True, stop=True,
                                 perf_mode=mybir.MatmulPerfMode.DoubleRow)

            # Dense fp8 DoubleRow matmul: lhsT = xs (stationary), rhs = wq.
            # 2-bank PSUM half-units (bufs=4) release banks mid-mi so the
            # copy+store chain hides under the next unit's matmuls.
            # Task = (mi, o-half) with its own 2-bank PSUM unit (bufs=4).
            n_units = max(OT // 2, 1)
            bpu = OT // n_units
            uw = bpu * 512
            ramp4 = MT >= 16 and n_units == 2
            ot_tiles: dict = {}
            tasks_left = {mi: n_units for mi in range(MT)}

            def new_ot(mi):
                ot_tiles[mi] = out_pool.tile(
                    [128, o_dim], mybir.dt.float16, name="ot", tag="ot")
                return ot_tiles[mi]

            def mm_unit(ps, mi, half, kc):
                lhsT = xs_sb[:, mi, kc].bitcast(
                    mybir.dt.float8e4)                        # [128,2,128]
                for oi in range(bpu):
                    o0 = (bpu * half + oi) * 512
                    rhs = wq_sb[:, kc, :, o0:o0 + 512]        # [128,2,512]
                    nc.tensor.matmul(
                        ps[:, bass.ts(oi, 512)], lhsT, rhs,
                        start=(kc == 0), stop=(kc == KC - 1),
                        perf_mode=mybir.MatmulPerfMode.DoubleRow)

            def finish(mi, half, ps):
                # psum -> sbuf, f32 -> f16 (exact); alternate DVE / ACT
                dst = ot_tiles[mi][:, half * uw:(half + 1) * uw]
                if (mi + half) % 2 == 0:
                    nc.vector.tensor_copy(dst, ps)
                else:
                    nc.scalar.copy(dst, ps)
                tasks_left[mi] -= 1
                if tasks_left[mi] == 0:
                    # one 0.5 MB store per mi on the SP HWDGE queue
                    nc.sync.dma_start(
                        out=out[bass.ts(mi, 128)], in_=ot_tiles[mi])

            tasks = [(mi, h) for mi in range(MT) for h in range(n_units)]

            for mi, half in tasks:
                if mi not in ot_tiles:
                    new_ot(mi)
                ps = psum_pool.tile([128, uw], mybir.dt.float32,
                                    name="ps", tag="ps")
                for kc in range(KC):
                    mm_unit(ps, mi, half, kc)
                finish(mi, half, ps)

    # run_bass_via_pjrt does not finalize prebuilt modules; Bacc.finalize()
    # runs compile() (event-semaphore wait splitting, reg alloc, fusion).
    nc.finalize()
    return nc


def ternarize_host(weight: np.ndarray) -> np.ndarray:
    """absmean ternarization, f64 for a faithful gamma; returns {-1,0,1} f32."""
    w = weight.astype(np.float64)
    gamma = np.mean(np.abs(w)) + EPS
    return (np.sign(w) * np.minimum(np.round(np.abs(w) / gamma), 1.0)).astype(
        np.float32)


def _pack_kpj(a_t: np.ndarray) -> np.ndarray:
    """[k_dim, n] -> [KC, 128, 2, n] with i = kc*256 + j*128 + p."""
    k_dim, n = a_t.shape
    return np.ascontiguousarray(
        a_t.reshape(k_dim // 256, 2, 128, n).transpose(0, 2, 1, 3))


def pack_x_flat(x_t: np.ndarray) -> np.ndarray:
    """x^T hi-bytes [k_dim, m] -> flat u8 per-group partition-major blocks."""
    k_dim, m = x_t.shape
    # [mb, p, kc, j, mi]
    a = x_t.reshape(k_dim // 256, 2, 128, m // 128, 128).transpose(3, 2, 0, 1, 4)
    blocks = [np.ascontiguousarray(a[b0:b1].transpose(1, 0, 2, 3, 4)).reshape(-1)
              for b0, b1 in _x_groups(m // 128)]
    return np.concatenate(blocks)


def pack_w_flat(wq_t: np.ndarray) -> np.ndarray:
    """ternary Wq^T [k_dim, o] f32 -> flat u8 (fp8e4 bytes), grouped."""
    w4 = _pack_kpj(wq_t).astype(FP8).view(np.uint8)  # [KC, 128, 2, o]
    blocks = [
        np.ascontiguousarray(
            w4[k0:k1, :, :, c0:c1].transpose(1, 0, 2, 3)).reshape(-1)
        for k0, k1, c0, c1 in _w_groups(w4.shape[0], w4.shape[3])]
    return np.concatenate(blocks)


def prep_in_maps(x: np.ndarray, weight: np.ndarray) -> list[dict]:
    wq = ternarize_host(weight)                    # [o, i] ternary
    wt = pack_w_flat(np.ascontiguousarray(wq.T))
    xf = x.reshape(M_TOT, I_DIM)
    in_maps = []
    for c in range(N_CORES):
        sh = xf[c * M_PER:(c + 1) * M_PER]         # [m_per, i]
        xb = np.ascontiguousarray(sh.T).astype(ml_dtypes.bfloat16)  # [i, m]
        # high byte of bf16: sign + 7 exponent bits — all sign() needs
        hi = (xb.view(np.uint16) >> 8).astype(np.uint8)             # [i, m]
        in_maps.append({"xt": pack_x_flat(hi), "wt": wt})
    return in_maps


_PROGRAM_CACHE: dict = {}


def _get_program() -> bass.Bass:
    key = (M_PER, I_DIM, O_DIM)
    if key not in _PROGRAM_CACHE:
        _PROGRAM_CACHE[key] = build_program(*key)
    return _PROGRAM_CACHE[key]


def _gather(results: list[dict]) -> np.ndarray:
    full = np.concatenate([np.asarray(r["out"]) for r in results], axis=0)
    return np.ascontiguousarray(full.reshape(B, S, O_DIM).astype(np.float32))


def kernel(x: np.ndarray, weight: np.ndarray) -> np.ndarray:
    nc = _get_program()
    in_maps = prep_in_maps(np.asarray(x), np.asarray(weight))
    res = run_bass_kernel_spmd(nc, in_maps, core_ids=list(range(N_CORES)))
    return _gather(res.results)


def kernel_traced(x: np.ndarray, weight: np.ndarray, **trace_kw):
    """Like kernel() but returns (output, BassKernelResults) with a trace."""
    nc = _get_program()
    in_maps = prep_in_maps(np.asarray(x), np.asarray(weight))
    res = run_bass_kernel_spmd(
        nc, in_maps, core_ids=list(range(N_CORES)), trace=True, **trace_kw)
    return _gather(res.results), res



# revision 2
# speedup vs baseline: 1.0692x; 1.0692x over previous
"""BitLinear (1.58-bit) kernel for Trainium2, 8-core data-parallel SPMD.

Reference op: out = sign(x) @ ternarize(W).T where
  ternarize(W) = sign(W) * min(round(|W| / gamma), 1), gamma = mean(|W|) + 1e-6.

Strategy (per sharding hint: data-parallel over batch*seq, replicate ternary W):
  - Host: ternarize W once, transpose to [in, out], pack as fp8e4 bytes
    (exact for -1/0/+1).  Shard x by rows across 8 cores; send only the SIGN
    BITS of each x shard (8 contraction-slots per byte) - 0.5 MB per core
    instead of 4.2 MB, so input DMA never starves the PE.
  - Device (per core): expand sign bits to fp8 {+1,-1} bytes on DVE
    (shift/and then or-in the fp8 exponent bits), then dense fp8 DoubleRow
    matmuls (2 MACs/cell/cyc) accumulating in PSUM f32.  Products are +-1 and
    row sums <= 2048 so fp32 accumulation is exact.
  - Host: concatenate + re-tile the 8 per-core outputs.

Schedule: the 16x4 (m-tile x o-quarter) unit grid runs q-outer within
mi-blocks of 8, so each 1 MB weight quarter is first needed ~14 us after the
previous one (vs. all 4.2 MB inside the first 7 us for mi-major order).  Each
unit accumulates all 8 contraction chunks into one PSUM bank, copies to SBUF
f16 (alternating DVE/ACT), and stores its own contiguous 128 KB DRAM block -
spreading output traffic evenly and shrinking the kernel tail.

Layout: contraction index i in [0, 2048) is split as i = kc*256 + j*128 + p
(kc = 256-wide chunk, j = DoubleRow pair slot, p = SBUF partition).  Both
matmul operands are stored [128, ..., 2, N] in SBUF and sliced to the 3D
[128 part, 2, N] APs that MatmulPerfMode.DoubleRow requires.
"""

import numpy as np
import ml_dtypes

import concourse.bass as bass
import concourse.bacc as bacc
import concourse.mybir as mybir
from concourse.tile import TileContext
from concourse.bass_utils import run_bass_kernel_spmd

FP8 = ml_dtypes.float8_e4m3  # maps to mybir.dt.float8e4

N_CORES = 8
EPS = 1e-6

# Full-problem shapes (hardcoded per harness contract).
B, S, I_DIM, O_DIM = 4, 4096, 2048, 2048
M_TOT = B * S                 # 16384 rows
M_PER = M_TOT // N_CORES      # 2048 rows per core

KC = I_DIM // 256             # 8 contraction chunks
MT = M_PER // 128             # 16 output row tiles
QT = O_DIM // 512             # 4 output col quarters (one PSUM bank each)
MI_BLK = 8                    # m-tiles per schedule block (q-outer inside)

# x sign-bit DMA groups (mi ranges) and weight DMA groups (kc-range, quarter),
# both deadline-ordered.
X_GROUPS = [(0, 2), (2, 8), (8, MT)]
W_GROUPS = [(0, 1, 0), (1, 2, 0), (2, 4, 0), (4, 8, 0),
            (0, 8, 1), (0, 8, 2), (0, 8, 3)]


def build_program() -> bass.Bass:
    """Per-core SPMD program: out[m, o] = sign(x)[m, :] @ Wq[o, :].T.

    DRAM inputs (flat u8, concatenated per-DMA-group partition-major blocks):
      xp : sign bits of x^T, byte [p, mi, j, m] holds bits kc=0..7
           (bit kc = 1 iff x < 0), i = kc*256 + j*128 + p, m = mi*128 + m'
      wt : ternary Wq^T as fp8e4 bytes, blocks [128p, kcr, 2, 512]
    DRAM output:
      out: [MT*QT*128, 512] f16; block (mi*QT + q) holds rows mi*128..+128,
           cols q*512..+512 (host re-tiles; integer values <= 2048, exact)
    """
    nc = bacc.Bacc()

    xp_total = 128 * MT * 2 * 128
    w_total = KC * 128 * 2 * O_DIM
    xp = nc.declare_dram_parameter(
        "xp", [xp_total], mybir.dt.uint8, isOutput=False)
    wt = nc.declare_dram_parameter(
        "wt", [w_total], mybir.dt.uint8, isOutput=False)
    out = nc.declare_dram_parameter(
        "out", [MT * QT * 128, 512], mybir.dt.float16, isOutput=True)

    with TileContext(nc) as tc:
        with (
            tc.tile_pool(name="wq", bufs=1) as wq_pool,
            tc.tile_pool(name="xs", bufs=1) as xs_pool,
            tc.tile_pool(name="xpk", bufs=1) as xp_pool,
            tc.tile_pool(name="psum", bufs=6, space="PSUM") as psum_pool,
            tc.tile_pool(name="osb", bufs=6) as out_pool,
        ):
            xp_sb = xp_pool.tile([128, MT, 2, 128], mybir.dt.uint8)
            xs_sb = xs_pool.tile([128, MT, KC, 2, 128], mybir.dt.float8e4)
            wq_sb = wq_pool.tile([128, KC, 2, O_DIM], mybir.dt.float8e4)

            # Every DMA group is a contiguous flat slice (per-group
            # partition-major host packing), deadline-ordered:
            #   SP queue:  xp mi0-1 | xp mi2-7 | xp mi8-15 | 64 output stores
            #   ACT queue: w (q0,kc0) | (q0,kc1) | (q0,kc2-3) | (q0,kc4-7)
            #              | (q1) | (q2) | (q3)
            x_off = {}
            off = 0
            for b0, b1 in X_GROUPS:
                x_off[(b0, b1)] = off
                off += 128 * (b1 - b0) * 2 * 128
            w_off = {}
            off = 0
            for g in W_GROUPS:
                w_off[g] = off
                off += 128 * (g[1] - g[0]) * 2 * 512

            for b0, b1 in X_GROUPS:
                sz = 128 * (b1 - b0) * 2 * 128
                o0 = x_off[(b0, b1)]
                nc.sync.dma_start(
                    out=xp_sb[:, b0:b1],
                    in_=xp[o0:o0 + sz].rearrange("(p r) -> p r", p=128))
            for g in W_GROUPS:
                k0, k1, q = g
                sz = 128 * (k1 - k0) * 2 * 512
                o0 = w_off[g]
                nc.scalar.dma_start(
                    out=wq_sb[:, k0:k1, :, q * 512:(q + 1) * 512].bitcast(
                        mybir.dt.uint8),
                    in_=wt[o0:o0 + sz].rearrange("(p r) -> p r", p=128))

            # PE warmup: dummy matmuls on memset scratch keep the PE busy
            # through the HAM activity window while the first chunks land,
            # so real matmuls start at the 2.4 GHz warm clock.
            wu_a = wq_pool.tile([128, 2, 128], mybir.dt.float8e4)
            wu_b = wq_pool.tile([128, 2, 512], mybir.dt.float8e4)
            nc.gpsimd.memset(wu_a, 0.0)
            nc.gpsimd.memset(wu_b, 0.0)
            wu_ps = psum_pool.tile([128, 512], mybir.dt.float32,
                                   name="wu_ps", tag="ps")
            for _ in range(10):
                nc.tensor.matmul(wu_ps, wu_a, wu_b, start=True, stop=True,
                                 perf_mode=mybir.MatmulPerfMode.DoubleRow)

            xs_u32 = xs_sb.bitcast(mybir.dt.uint32)
            xp_u32 = xp_sb.bitcast(mybir.dt.uint32)

            def expand_x(mi):
                # Sign bits -> fp8 {+1,-1}: bit kc shifted to each byte's MSB
                # (fp8 sign bit), then OR in 0x38 (the fp8e4 encoding of 1.0).
                for kc in range(KC):
                    nc.vector.tensor_scalar(
                        out=xs_u32[:, mi, kc], in0=xp_u32[:, mi],
                        scalar1=7 - kc, scalar2=0x80808080,
                        op0=mybir.AluOpType.logical_shift_left,
                        op1=mybir.AluOpType.bitwise_and)
                nc.vector.tensor_scalar(
                    out=xs_u32[:, mi], in0=xs_u32[:, mi],
                    scalar1=0x38383838, scalar2=None,
                    op0=mybir.AluOpType.bitwise_or)

            # Dense fp8 DoubleRow matmuls: lhsT = xs[mi, kc] (stationary),
            # rhs = wq[kc, q-slice].  One PSUM bank per unit, 8-chunk
            # accumulation, then f32 -> f16 copy (exact) and a contiguous
            # 128 KB store per unit.
            expanded = set()
            unit = 0
            for blk0 in range(0, MT, MI_BLK):
                for q in range(QT):
                    for mi in range(blk0, blk0 + MI_BLK):
                        if mi not in expanded:
                            expand_x(mi)
                            expanded.add(mi)
                        ps = psum_pool.tile([128, 512], mybir.dt.float32,
                                            name="ps", tag="ps")
                        for kc in range(KC):
                            nc.tensor.matmul(
                                ps, xs_sb[:, mi, kc],
                                wq_sb[:, kc, :, q * 512:(q + 1) * 512],
                                start=(kc == 0), stop=(kc == KC - 1),
                                perf_mode=mybir.MatmulPerfMode.DoubleRow)
                        ot = out_pool.tile([128, 512], mybir.dt.float16,
                                           name="ot", tag="ot")
                        if unit % 2 == 0:
                            nc.vector.tensor_copy(ot, ps)
                        else:
                            nc.scalar.copy(ot, ps)
                        nc.sync.dma_start(
                            out=out[bass.ts(mi * QT + q, 128)], in_=ot)
                        unit += 1

    nc.finalize()
    return nc


def ternarize_host(weight: np.ndarray) -> np.ndarray:
    """absmean ternarization, f64 for a faithful gamma; returns {-1,0,1} f32."""
    w = weight.astype(np.float64)
    gamma = np.mean(np.abs(w)) + EPS
    return (np.sign(w) * np.minimum(np.round(np.abs(w) / gamma), 1.0)).astype(
        np.float32)


def pack_w_flat(wq_t: np.ndarray) -> np.ndarray:
    """ternary Wq^T [i, o] f32 -> flat u8 (fp8e4 bytes), DMA-grouped."""
    # [kc, j, p, o] -> fp8 bytes
    w4 = wq_t.reshape(KC, 2, 128, O_DIM).astype(FP8).view(np.uint8)
    blocks = []
    for k0, k1, q in W_GROUPS:
        blk = w4[k0:k1, :, :, q * 512:(q + 1) * 512]     # [kcr, 2, 128, 512]
        blocks.append(np.ascontiguousarray(
            blk.transpose(2, 0, 1, 3)).reshape(-1))      # partition-major
    return np.concatenate(blocks)


def pack_x_flat(sh: np.ndarray) -> np.ndarray:
    """x shard [m_per, i] f32 -> flat u8 sign-bit planes, DMA-grouped.

    Byte (p, mi, j, m) holds bit kc = signbit(x[mi*128+m, kc*256+j*128+p]).
    """
    sb = np.signbit(sh)                                   # [m, i] bool
    # [kc, j, p, mi, m] -> [p, mi, j, m, kc]
    b = sb.T.reshape(KC, 2, 128, MT, 128).transpose(2, 3, 1, 4, 0)
    pk = np.packbits(np.ascontiguousarray(b), axis=-1,
                     bitorder="little")[..., 0]           # [128, MT, 2, 128]
    blocks = [np.ascontiguousarray(pk[:, b0:b1]).reshape(-1)
              for b0, b1 in X_GROUPS]
    return np.concatenate(blocks)


def prep_in_maps(x: np.ndarray, weight: np.ndarray) -> list[dict]:
    wq = ternarize_host(weight)                    # [o, i] ternary
    wt = pack_w_flat(np.ascontiguousarray(wq.T))
    xf = x.reshape(M_TOT, I_DIM)
    return [{"xp": pack_x_flat(xf[c * M_PER:(c + 1) * M_PER]), "wt": wt}
            for c in range(N_CORES)]


_PROGRAM_CACHE: dict = {}


def _get_program() -> bass.Bass:
    if "nc" not in _PROGRAM_CACHE:
        _PROGRAM_CACHE["nc"] = build_program()
    return _PROGRAM_CACHE["nc"]


def _gather(results: list[dict]) -> np.ndarray:
    # per-core out [MT*QT*128, 512] -> [m_per, o]
    shards = [
        np.asarray(r["out"]).reshape(MT, QT, 128, 512)
        .transpose(0, 2, 1, 3).reshape(M_PER, O_DIM)
        for r in results]
    full = np.concatenate(shards, axis=0)
    return np.ascontiguousarray(full.reshape(B, S, O_DIM).astype(np.float32))


def kernel(x: np.ndarray, weight: np.ndarray) -> np.ndarray:
    nc = _get_program()
    in_maps = prep_in_maps(np.asarray(x), np.asarray(weight))
    res = run_bass_kernel_spmd(nc, in_maps, core_ids=list(range(N_CORES)))
    return _gather(res.results)


def kernel_traced(x: np.ndarray, weight: np.ndarray, **trace_kw):
    """Like kernel() but returns (output, BassKernelResults) with a trace."""
    nc = _get_program()
    in_maps = prep_in_maps(np.asarray(x), np.asarray(weight))
    res = run_bass_kernel_spmd(
        nc, in_maps, core_ids=list(range(N_CORES)), trace=True, **trace_kw)
    return _gather(res.results), res


# revision 8
# speedup vs baseline: 1.0724x; 1.0030x over previous
"""BitLinear (1.58-bit) kernel for Trainium2, 8-core data-parallel SPMD.

Reference op: out = sign(x) @ ternarize(W).T where
  ternarize(W) = sign(W) * min(round(|W| / gamma), 1), gamma = mean(|W|) + 1e-6.

Strategy (per sharding hint: data-parallel over batch*seq, replicate ternary W):
  - Host: ternarize W once, transpose to [in, out], pack as fp8e4 bytes
    (exact for -1/0/+1).  Shard x by rows across 8 cores; send only the SIGN
    BITS of each x shard (8 contraction-slots per byte) - 0.5 MB per core
    instead of 4.2 MB, so input DMA never starves the PE.
  - Device (per core): expand sign bits to fp8 {+1,-1} bytes on DVE
    (shift/and then or-in the fp8 exponent bits), then dense fp8 DoubleRow
    matmuls (2 MACs/cell/cyc) accumulating in PSUM f32.  Products are +-1 and
    row sums <= 2048 so fp32 accumulation is exact.
  - Host: concatenate + re-tile the 8 per-core outputs.

Schedule: the 16x4 (m-tile x o-quarter) unit grid runs q-outer within
mi-blocks of 8, so each 1 MB weight quarter is first needed ~14 us after the
previous one (vs. all 4.2 MB inside the first 7 us for mi-major order).  Each
unit accumulates all 8 contraction chunks into one PSUM bank, copies to SBUF
f16 (alternating DVE/ACT), and stores its own contiguous 128 KB DRAM block -
spreading output traffic evenly and shrinking the kernel tail.

Layout: contraction index i in [0, 2048) is split as i = kc*256 + j*128 + p
(kc = 256-wide chunk, j = DoubleRow pair slot, p = SBUF partition).  Both
matmul operands are stored [128, ..., 2, N] in SBUF and sliced to the 3D
[128 part, 2, N] APs that MatmulPerfMode.DoubleRow requires.
"""

import numpy as np
import ml_dtypes

import concourse.bass as bass
import concourse.bacc as bacc
import concourse.mybir as mybir
from concourse.tile import TileContext
from concourse.bass_utils import run_bass_kernel_spmd

FP8 = ml_dtypes.float8_e4m3  # maps to mybir.dt.float8e4

N_CORES = 8
EPS = 1e-6

# Full-problem shapes (hardcoded per harness contract).
B, S, I_DIM, O_DIM = 4, 4096, 2048, 2048
M_TOT = B * S                 # 16384 rows
M_PER = M_TOT // N_CORES      # 2048 rows per core

KC = I_DIM // 256             # 8 contraction chunks
MT = M_PER // 128             # 16 output row tiles
QT = O_DIM // 512             # 4 output col quarters (one PSUM bank each)
MI_BLK = 4                    # m-tiles per schedule block (q-outer inside)

# x sign-bit DMA groups (mi ranges), weight DMA groups (kc-range, quarter),
# and x-expansion groups (mi ranges), all deadline-ordered.  Weight groups are
# spread over three otherwise-idle HWDGE queues so their ~1.4 us per-DMA
# receipt latencies overlap instead of serializing ahead of the first matmul.
X_GROUPS = [(0, 1), (1, 2), (2, 8), (8, MT)]
W_GROUPS = [(0, 2, 0), (2, 4, 0), (4, 8, 0),
            (0, 8, 1), (0, 8, 2), (0, 8, 3)]
E_GROUPS = [(0, 1), (1, 2), (2, 4), (4, 8), (8, 12), (12, MT)]


def build_program() -> bass.Bass:
    """Per-core SPMD program: out[m, o] = sign(x)[m, :] @ Wq[o, :].T.

    DRAM inputs (flat u8, concatenated per-DMA-group partition-major blocks):
      xp : sign bits of x^T, byte [p, mi, j, m] holds bits kc=0..7
           (bit kc = 1 iff x < 0), i = kc*256 + j*128 + p, m = mi*128 + m'
      wt : ternary Wq^T as fp8e4 bytes, blocks [128p, kcr, 2, 512]
    DRAM output:
      out: [MT*QT*128, 512] f16; block (mi*QT + q) holds rows mi*128..+128,
           cols q*512..+512 (host re-tiles; integer values <= 2048, exact)
    """
    nc = bacc.Bacc()

    xp_total = 128 * MT * 2 * 128
    w_total = KC * 128 * 2 * O_DIM
    xp = nc.declare_dram_parameter(
        "xp", [xp_total], mybir.dt.uint8, isOutput=False)
    wt = nc.declare_dram_parameter(
        "wt", [w_total], mybir.dt.uint8, isOutput=False)
    out = nc.declare_dram_parameter(
        "out", [MT * QT * 128, 512], mybir.dt.float16, isOutput=True)

    with TileContext(nc) as tc:
        with (
            tc.tile_pool(name="wq", bufs=1) as wq_pool,
            tc.tile_pool(name="xs", bufs=1) as xs_pool,
            tc.tile_pool(name="xpk", bufs=1) as xp_pool,
            tc.tile_pool(name="psum", bufs=8, space="PSUM") as psum_pool,
            tc.tile_pool(name="osb", bufs=8) as out_pool,
        ):
            xp_sb = xp_pool.tile([128, MT, 2, 128], mybir.dt.uint8)
            xs_sb = xs_pool.tile([128, MT, KC, 2, 128], mybir.dt.float8e4)
            wq_sb = wq_pool.tile([128, KC, 2, O_DIM], mybir.dt.float8e4)

            # Every DMA group is a contiguous flat slice (per-group
            # partition-major host packing), deadline-ordered:
            #   SP queue:   xp mi0 | mi1 | mi2-7 | mi8-15 | 64 output stores
            #   ACT queue:  w (q0,kc0-1) | (q0,kc2-3) | 64 psum copies
            #   POOL queue: warmup memsets | w (q0,kc4-7) | (q1) | (q2) | (q3)
            x_off = {}
            off = 0
            for b0, b1 in X_GROUPS:
                x_off[(b0, b1)] = off
                off += 128 * (b1 - b0) * 2 * 128
            w_off = {}
            off = 0
            for g in W_GROUPS:
                w_off[g] = off
                off += 128 * (g[1] - g[0]) * 2 * 512

            # Warmup scratch memsets go first on the POOL queue so the
            # blocking weight-DMA issues behind them cannot delay warmup.
            wu_a = wq_pool.tile([128, 2, 128], mybir.dt.float8e4)
            wu_b = wq_pool.tile([128, 2, 512], mybir.dt.float8e4)
            nc.gpsimd.memset(wu_a, 0.0)
            nc.gpsimd.memset(wu_b, 0.0)

            for b0, b1 in X_GROUPS:
                sz = 128 * (b1 - b0) * 2 * 128
                o0 = x_off[(b0, b1)]
                nc.sync.dma_start(
                    out=xp_sb[:, b0:b1],
                    in_=xp[o0:o0 + sz].rearrange("(p r) -> p r", p=128))
            w_eng = [nc.scalar, nc.scalar, nc.gpsimd,
                     nc.gpsimd, nc.gpsimd, nc.gpsimd]
            for g, eng in zip(W_GROUPS, w_eng):
                k0, k1, q = g
                sz = 128 * (k1 - k0) * 2 * 512
                o0 = w_off[g]
                eng.dma_start(
                    out=wq_sb[:, k0:k1, :, q * 512:(q + 1) * 512].bitcast(
                        mybir.dt.uint8),
                    in_=wt[o0:o0 + sz].rearrange("(p r) -> p r", p=128))

            # PE warmup: dummy matmuls on memset scratch keep the PE busy
            # through the HAM activity window while the first chunks land,
            # so real matmuls start at the 2.4 GHz warm clock.
            wu_ps = psum_pool.tile([128, 512], mybir.dt.float32,
                                   name="wu_ps", tag="ps")
            for _ in range(9):
                nc.tensor.matmul(wu_ps, wu_a, wu_b, start=True, stop=True,
                                 perf_mode=mybir.MatmulPerfMode.DoubleRow)

            xs_u32 = xs_sb.bitcast(mybir.dt.uint32)
            xp_u32 = xp_sb.bitcast(mybir.dt.uint32)

            def expand_x(m0, m1):
                # Sign bits -> fp8 {+1,-1}: bit kc shifted to each byte's MSB
                # (fp8 sign bit), then OR in 0x38 (the fp8e4 encoding of 1.0).
                # DVE instruction overhead is ~155 ns, so later groups batch
                # several mi per instruction; the first ones stay small to
                # unblock the matmul stream as early as possible.
                for kc in range(KC):
                    nc.vector.tensor_scalar(
                        out=xs_u32[:, m0:m1, kc], in0=xp_u32[:, m0:m1],
                        scalar1=7 - kc, scalar2=0x80808080,
                        op0=mybir.AluOpType.logical_shift_left,
                        op1=mybir.AluOpType.bitwise_and)
                nc.vector.tensor_scalar(
                    out=xs_u32[:, m0:m1], in0=xs_u32[:, m0:m1],
                    scalar1=0x38383838, scalar2=None,
                    op0=mybir.AluOpType.bitwise_or)

            # The whole expansion schedule is emitted up front: DVE has no
            # other work, so it runs the groups back-to-back, each gated only
            # by its xp DMA group.
            for m0, m1 in E_GROUPS:
                expand_x(m0, m1)

            # Dense fp8 DoubleRow matmuls: lhsT = xs[mi, kc] (stationary),
            # rhs = wq[kc, q-slice].  One PSUM bank per unit, 8-chunk
            # accumulation, then an f32 -> f16 ACT copy (exact) and a
            # contiguous 128 KB store per unit.
            for blk0 in range(0, MT, MI_BLK):
                for q in range(QT):
                    for mi in range(blk0, blk0 + MI_BLK):
                        ps = psum_pool.tile([128, 512], mybir.dt.float32,
                                            name="ps", tag="ps")
                        for kc in range(KC):
                            nc.tensor.matmul(
                                ps, xs_sb[:, mi, kc],
                                wq_sb[:, kc, :, q * 512:(q + 1) * 512],
                                start=(kc == 0), stop=(kc == KC - 1),
                                perf_mode=mybir.MatmulPerfMode.DoubleRow)
                        ot = out_pool.tile([128, 512], mybir.dt.float16,
                                           name="ot", tag="ot")
                        nc.scalar.copy(ot, ps)
                        nc.sync.dma_start(
                            out=out[bass.ts(mi * QT + q, 128)], in_=ot)

    nc.finalize()
    return nc


def ternarize_host(weight: np.ndarray) -> np.ndarray:
    """absmean ternarization, f64 for a faithful gamma; returns {-1,0,1} f32."""
    w = weight.astype(np.float64)
    gamma = np.mean(np.abs(w)) + EPS
    return (np.sign(w) * np.minimum(np.round(np.abs(w) / gamma), 1.0)).astype(
        np.float32)


def pack_w_flat(wq_t: np.ndarray) -> np.ndarray:
    """ternary Wq^T [i, o] f32 -> flat u8 (fp8e4 bytes), DMA-grouped."""
    # [kc, j, p, o] -> fp8 bytes
    w4 = wq_t.reshape(KC, 2, 128, O_DIM).astype(FP8).view(np.uint8)
    blocks = []
    for k0, k1, q in W_GROUPS:
        blk = w4[k0:k1, :, :, q * 512:(q + 1) * 512]     # [kcr, 2, 128, 512]
        blocks.append(np.ascontiguousarray(
            blk.transpose(2, 0, 1, 3)).reshape(-1))      # partition-major
    return np.concatenate(blocks)


def pack_x_flat(sh: np.ndarray) -> np.ndarray:
    """x shard [m_per, i] f32 -> flat u8 sign-bit planes, DMA-grouped.

    Byte (p, mi, j, m) holds bit kc = signbit(x[mi*128+m, kc*256+j*128+p]).
    """
    sb = np.signbit(sh)                                   # [m, i] bool
    # [kc, j, p, mi, m] -> [p, mi, j, m, kc]
    b = sb.T.reshape(KC, 2, 128, MT, 128).transpose(2, 3, 1, 4, 0)
    pk = np.packbits(np.ascontiguousarray(b), axis=-1,
                     bitorder="little")[..., 0]           # [128, MT, 2, 128]
    blocks = [np.ascontiguousarray(pk[:, b0:b1]).reshape(-1)
              for b0, b1 in X_GROUPS]
    return np.concatenate(blocks)


def prep_in_maps(x: np.ndarray, weight: np.ndarray) -> list[dict]:
    wq = ternarize_host(weight)                    # [o, i] ternary
    wt = pack_w_flat(np.ascontiguousarray(wq.T))
    xf = x.reshape(M_TOT, I_DIM)
    return [{"xp": pack_x_flat(xf[c * M_PER:(c + 1) * M_PER]), "wt": wt}
            for c in range(N_CORES)]


_PROGRAM_CACHE: dict = {}


def _get_program() -> bass.Bass:
    if "nc" not in _PROGRAM_CACHE:
        _PROGRAM_CACHE["nc"] = build_program()
    return _PROGRAM_CACHE["nc"]


def _gather(results: list[dict]) -> np.ndarray:
    # per-core out [MT*QT*128, 512] -> [m_per, o]
    shards = [
        np.asarray(r["out"]).reshape(MT, QT, 128, 512)
        .transpose(0, 2, 1, 3).reshape(M_PER, O_DIM)
        for r in results]
    full = np.concatenate(shards, axis=0)
    return np.ascontiguousarray(full.reshape(B, S, O_DIM).astype(np.float32))


def kernel(x: np.ndarray, weight: np.ndarray) -> np.ndarray:
    nc = _get_program()
    in_maps = prep_in_maps(np.asarray(x), np.asarray(weight))
    res = run_bass_kernel_spmd(nc, in_maps, core_ids=list(range(N_CORES)))
    return _gather(res.results)


def kernel_traced(x: np.ndarray, weight: np.ndarray, **trace_kw):
    """Like kernel() but returns (output, BassKernelResults) with a trace."""
    nc = _get_program()
    in_maps = prep_in_maps(np.asarray(x), np.asarray(weight))
    res = run_bass_kernel_spmd(
        nc, in_maps, core_ids=list(range(N_CORES)), trace=True, **trace_kw)
    return _gather(res.results), res
